# revision 10
# baseline (speedup 1.0000x reference)
"""Binarize kernel for Trainium2, 8-core data-parallel.

out[b, f] = 1.0 if (medians[f] > 0) and (x[b, f] >= medians[f]) else 0.0

Sharding: pure data parallel - x is split row-wise across the 8 NeuronCores
(2048 rows each); the 4096-entry medians vector is replicated.

Per-core device kernel (raw bass, three engine streams):
  * ACT ring: broadcast-DMA medians across the 128 partitions once, then
    stream the 16 output tiles back to HBM.
  * SP ring: stream the 16 [128, 4096] x tiles HBM->SBUF (starts at t=0).
  * DVE: mprime[f] = medians[f] if medians[f] > 0 else 3e38 (two prep ops),
    then one in-place is_ge compare per tile: xt = (xt >= mprime) -> 1.0/0.0.
    A single compare is exact - no arithmetic rounding anywhere.

Raw bass instead of the Tile framework because walrus codegen allows only a
single sync-wait command on a compute instruction; all waits here are
standalone queue commands. Each of the NBUF=8 buffer slots has its own
load/store semaphore pair: increments on one semaphore are serialized by the
slot's dependency chain, so count thresholds are race-free even though DMA
completions across slots may reorder. The kernel is HBM-bound: ~64 MiB of
HBM traffic per core at ~330 GB/s measured => ~205 us steady-state.

reps > 1 re-runs the identical pipeline inside one NEFF (slope-based HW
timing); the output is unchanged.
"""

import contextlib

import numpy as np

import concourse.bass as bass
import concourse.mybir as mybir
from concourse.bass_utils import run_bass_kernel_spmd

N_CORES = 8
B_FULL = 16384
F = 4096
ROWS = B_FULL // N_CORES  # 2048 rows per core
P = 128
N_TILES = ROWS // P  # 16
NBUF = 8

_BIG = 3.0e38  # pushes the compare threshold above any finite fp32 input


def _build_nc(reps: int = 1) -> bass.Bass:
    nc = bass.Bass()
    dt = mybir.dt.float32
    x = nc.dram_tensor("x", [ROWS, F], dt, kind="ExternalInput")
    med = nc.dram_tensor("med", [F], dt, kind="ExternalInput")
    out = nc.dram_tensor("out", [ROWS, F], dt, kind="ExternalOutput")

    x_t = x.rearrange("(n p) f -> n p f", p=P)
    o_t = out.rearrange("(n p) f -> n p f", p=P)
    med_b = med[None, :].broadcast_to((P, F))

    n_iters = reps * N_TILES

    with contextlib.ExitStack() as ctx:
        m_b = ctx.enter_context(nc.sbuf_tensor("m_b", [P, F], dt))
        mprime = ctx.enter_context(nc.sbuf_tensor("mprime", [P, F], dt))
        xt = ctx.enter_context(nc.sbuf_tensor("xt", [P, NBUF, F], dt))
        s_med = ctx.enter_context(nc.semaphore("s_med"))
        s_ld = [ctx.enter_context(nc.semaphore(f"s_ld{s}")) for s in range(NBUF)]
        s_st = [ctx.enter_context(nc.semaphore(f"s_st{s}")) for s in range(NBUF)]
        s_dve = ctx.enter_context(nc.semaphore("s_dve"))
        block = ctx.enter_context(nc.Block())

        # s_dve counts: +1 per mprime prep op (2), then +1 per TT_i,
        # so after TT_i the value is i + 3.

        @block.sync
        def _(sync):
            for i in range(n_iters):
                s = i % NBUF
                if i >= NBUF:
                    # overwriting xt[:, s]: store_{i-NBUF} done implies
                    # TT_{i-NBUF} done as well
                    sync.wait_ge(s_st[s], 16 * (i // NBUF))
                sync.dma_start(out=xt[:, s], in_=x_t[i % N_TILES]).then_inc(
                    s_ld[s], 16
                )

        @block.scalar
        def _(scalar):
            scalar.dma_start(out=m_b[:], in_=med_b).then_inc(s_med, 16)
            for i in range(n_iters):
                s = i % NBUF
                scalar.wait_ge(s_dve, i + 3)  # TT_i rewrote xt[:, s]
                scalar.dma_start(out=o_t[i % N_TILES], in_=xt[:, s]).then_inc(
                    s_st[s], 16
                )
            # all stores landed before the NEFF retires
            for s in range(NBUF):
                scalar.wait_ge(s_st[s], 16 * (n_iters // NBUF))

        @block.vector
        def _(vector):
            vector.wait_ge(s_med, 16)  # m_b present
            # mprime = (m_b <= 0) * BIG + m_b; sem handshakes order the
            # back-to-back DVE ops (same-engine RAW is not implicit)
            nc.vector.tensor_scalar(
                out=mprime[:],
                in0=m_b[:],
                scalar1=0.0,
                scalar2=_BIG,
                op0=mybir.AluOpType.is_le,
                op1=mybir.AluOpType.mult,
            ).then_inc(s_dve, 1)
            vector.wait_ge(s_dve, 1)
            nc.vector.tensor_add(out=mprime[:], in0=mprime[:], in1=m_b[:]).then_inc(
                s_dve, 1
            )
            vector.wait_ge(s_dve, 2)
            for i in range(n_iters):
                s = i % NBUF
                if i >= NBUF:
                    # in-place overwrite of xt[:, s] must wait until
                    # store_{i-NBUF} has read it
                    vector.wait_ge(s_st[s], 16 * (i // NBUF))
                vector.wait_ge(s_ld[s], 16 * (i // NBUF + 1))  # xt[:, s] loaded
                nc.vector.tensor_tensor(
                    out=xt[:, s], in0=xt[:, s], in1=mprime[:],
                    op=mybir.AluOpType.is_ge,
                ).then_inc(s_dve, 1)

    return nc


_NC_CACHE: list[bass.Bass] = []


def _get_nc() -> bass.Bass:
    if not _NC_CACHE:
        _NC_CACHE.append(_build_nc())
    return _NC_CACHE[0]


def kernel(x: np.ndarray, medians: np.ndarray) -> np.ndarray:
    x = np.ascontiguousarray(x, dtype=np.float32)
    medians = np.ascontiguousarray(medians, dtype=np.float32)
    assert x.shape == (B_FULL, F), x.shape
    assert medians.shape == (F,), medians.shape

    nc = _get_nc()
    in_maps = [
        {"x": x[c * ROWS : (c + 1) * ROWS], "med": medians} for c in range(N_CORES)
    ]
    res = run_bass_kernel_spmd(nc, in_maps, core_ids=list(range(N_CORES)))
    return np.concatenate([res.results[c]["out"] for c in range(N_CORES)], axis=0)


# revision 17
# speedup vs baseline: 1.0245x; 1.0245x over previous
"""Binarize kernel for Trainium2, 8-core data-parallel.

out[b, f] = 1.0 if (medians[f] > 0) and (x[b, f] >= medians[f]) else 0.0

Sharding: pure data parallel - x is split row-wise across the 8 NeuronCores
(2048 rows each); the 4096-entry medians vector is replicated.

Per-core device kernel (raw bass, three engine streams):
  * ACT ring: broadcast-DMA medians across the 128 partitions once, then
    stream the 16 output tiles back to HBM.
  * SP ring: stream the 16 [128, 4096] x tiles HBM->SBUF (starts at t=0).
  * DVE: mprime[f] = medians[f] if medians[f] > 0 else 3e38 (two prep ops),
    then one in-place is_ge compare per tile: xt = (xt >= mprime) -> 1.0/0.0.
    A single compare is exact - no arithmetic rounding anywhere.

Raw bass instead of the Tile framework because walrus codegen allows only a
single sync-wait command on a compute instruction; all waits here are
standalone queue commands. Each of the NBUF=8 buffer slots has its own
load/store semaphore pair: increments on one semaphore are serialized by the
slot's dependency chain, so count thresholds are race-free even though DMA
completions across slots may reorder. The kernel is HBM-bound: ~64 MiB of
HBM traffic per core at ~330 GB/s measured => ~205 us steady-state.

reps > 1 re-runs the identical pipeline inside one NEFF (slope-based HW
timing); the output is unchanged.
"""

import contextlib

import numpy as np

import concourse.bass as bass
import concourse.mybir as mybir
from concourse.bass_utils import run_bass_kernel_spmd

N_CORES = 8
B_FULL = 16384
F = 4096
ROWS = B_FULL // N_CORES  # 2048 rows per core
P = 128
N_TILES = ROWS // P  # 16
NBUF = 8

_BIG = 3.0e38  # pushes the compare threshold above any finite fp32 input


def _build_nc(reps: int = 1) -> bass.Bass:
    nc = bass.Bass()
    dt = mybir.dt.float32
    x = nc.dram_tensor("x", [ROWS, F], dt, kind="ExternalInput")
    med = nc.dram_tensor("med", [F], dt, kind="ExternalInput")
    out = nc.dram_tensor("out", [ROWS, F], dt, kind="ExternalOutput")

    x_t = x.rearrange("(n p) f -> n p f", p=P)
    o_t = out.rearrange("(n p) f -> n p f", p=P)

    n_iters = reps * N_TILES

    with contextlib.ExitStack() as ctx:
        m_b = ctx.enter_context(nc.sbuf_tensor("m_b", [P, F], dt))
        mprime = ctx.enter_context(nc.sbuf_tensor("mprime", [P, F], dt))
        xt = ctx.enter_context(nc.sbuf_tensor("xt", [P, NBUF, F], dt))
        s_med = ctx.enter_context(nc.semaphore("s_med"))
        s_bc = ctx.enter_context(nc.semaphore("s_bc"))
        s_ld = [ctx.enter_context(nc.semaphore(f"s_ld{s}")) for s in range(NBUF)]
        s_st = [ctx.enter_context(nc.semaphore(f"s_st{s}")) for s in range(NBUF)]
        s_dve = ctx.enter_context(nc.semaphore("s_dve"))
        block = ctx.enter_context(nc.Block())

        # s_dve counts: +1 per mprime prep op (2), then +1 per TT_i,
        # so after TT_i the value is i + 3.

        @block.sync
        def _(sync):
            for i in range(n_iters):
                s = i % NBUF
                if i >= NBUF:
                    # overwriting xt[:, s]: store_{i-NBUF} done implies
                    # TT_{i-NBUF} done as well
                    sync.wait_ge(s_st[s], 16 * (i // NBUF))
                sync.dma_start(out=xt[:, s], in_=x_t[i % N_TILES]).then_inc(
                    s_ld[s], 16
                )

        @block.scalar
        def _(scalar):
            # 16 KB medians row -> partition 0; prep runs on that row, then
            # log2 doubling copies spread mprime row 0 across all 128
            # partitions SBUF->SBUF (only 16 KB of HBM read instead of the
            # 2 MiB a DRAM-side broadcast would re-read)
            scalar.dma_start(out=m_b[:1, :], in_=med[None, :]).then_inc(s_med, 16)
            scalar.wait_ge(s_dve, 2)  # mprime[0:1, :] final
            k, chain = 1, 0
            while k < P:
                scalar.dma_start(
                    out=mprime[k : 2 * k, :], in_=mprime[:k, :]
                ).then_inc(s_bc, 16)
                chain += 1
                scalar.wait_ge(s_bc, 16 * chain)
                k *= 2
            for i in range(n_iters):
                s = i % NBUF
                scalar.wait_ge(s_dve, i + 3)  # TT_i rewrote xt[:, s]
                scalar.dma_start(out=o_t[i % N_TILES], in_=xt[:, s]).then_inc(
                    s_st[s], 16
                )
            # all stores landed before the NEFF retires
            for s in range(NBUF):
                scalar.wait_ge(s_st[s], 16 * (n_iters // NBUF))

        @block.vector
        def _(vector):
            vector.wait_ge(s_med, 16)  # medians row present
            # mprime = (med <= 0) * BIG + med, on partition 0 only; sem
            # handshakes order the back-to-back DVE ops (same-engine RAW is
            # not implicit)
            nc.vector.tensor_scalar(
                out=mprime[:1, :],
                in0=m_b[:1, :],
                scalar1=0.0,
                scalar2=_BIG,
                op0=mybir.AluOpType.is_le,
                op1=mybir.AluOpType.mult,
            ).then_inc(s_dve, 1)
            vector.wait_ge(s_dve, 1)
            nc.vector.tensor_add(
                out=mprime[:1, :], in0=mprime[:1, :], in1=m_b[:1, :]
            ).then_inc(s_dve, 1)
            vector.wait_ge(s_bc, 16 * 7)  # broadcast chain done (2^7 = 128)
            for i in range(n_iters):
                s = i % NBUF
                if i >= NBUF:
                    # in-place overwrite of xt[:, s] must wait until
                    # store_{i-NBUF} has read it
                    vector.wait_ge(s_st[s], 16 * (i // NBUF))
                vector.wait_ge(s_ld[s], 16 * (i // NBUF + 1))  # xt[:, s] loaded
                nc.vector.tensor_tensor(
                    out=xt[:, s], in0=xt[:, s], in1=mprime[:],
                    op=mybir.AluOpType.is_ge,
                ).then_inc(s_dve, 1)

    return nc


_NC_CACHE: list[bass.Bass] = []


def _get_nc() -> bass.Bass:
    if not _NC_CACHE:
        _NC_CACHE.append(_build_nc())
    return _NC_CACHE[0]


def kernel(x: np.ndarray, medians: np.ndarray) -> np.ndarray:
    x = np.ascontiguousarray(x, dtype=np.float32)
    medians = np.ascontiguousarray(medians, dtype=np.float32)
    assert x.shape == (B_FULL, F), x.shape
    assert medians.shape == (F,), medians.shape

    nc = _get_nc()
    in_maps = [
        {"x": x[c * ROWS : (c + 1) * ROWS], "med": medians} for c in range(N_CORES)
    ]
    res = run_bass_kernel_spmd(nc, in_maps, core_ids=list(range(N_CORES)))
    return np.concatenate([res.results[c]["out"] for c in range(N_CORES)], axis=0)


# revision 24
# speedup vs baseline: 1.0369x; 1.0121x over previous
"""Binarize kernel for Trainium2, 8-core data-parallel.

out[b, f] = 1.0 if (medians[f] > 0) and (x[b, f] >= medians[f]) else 0.0

Sharding: pure data parallel - x is split row-wise across the 8 NeuronCores
(2048 rows each); the 4096-entry medians vector is replicated.

Per-core device kernel (raw bass, three engine streams):
  * ACT ring: broadcast-DMA medians across the 128 partitions once, then
    stream the 16 output tiles back to HBM.
  * SP ring: stream the 16 [128, 4096] x tiles HBM->SBUF (starts at t=0).
  * DVE: mprime[f] = medians[f] if medians[f] > 0 else 3e38 (two prep ops),
    then one in-place is_ge compare per tile: xt = (xt >= mprime) -> 1.0/0.0.
    A single compare is exact - no arithmetic rounding anywhere.

Raw bass instead of the Tile framework because walrus codegen allows only a
single sync-wait command on a compute instruction; all waits here are
standalone queue commands. Each of the NBUF=8 buffer slots has its own
load/store semaphore pair: increments on one semaphore are serialized by the
slot's dependency chain, so count thresholds are race-free even though DMA
completions across slots may reorder. The kernel is HBM-bound: ~64 MiB of
HBM traffic per core at ~330 GB/s measured => ~205 us steady-state.

reps > 1 re-runs the identical pipeline inside one NEFF (slope-based HW
timing); the output is unchanged.
"""

import contextlib

import numpy as np

import concourse.bass as bass
import concourse.mybir as mybir
from concourse.bass_utils import run_bass_kernel_spmd

N_CORES = 8
B_FULL = 16384
F = 4096
ROWS = B_FULL // N_CORES  # 2048 rows per core
P = 128
N_TILES = ROWS // P  # 16
NBUF = 8

_BIG = 3.0e38  # pushes the compare threshold above any finite fp32 input


def _build_nc(reps: int = 1) -> bass.Bass:
    nc = bass.Bass()
    dt = mybir.dt.float32
    x = nc.dram_tensor("x", [ROWS, F], dt, kind="ExternalInput")
    med = nc.dram_tensor("med", [F], dt, kind="ExternalInput")
    out = nc.dram_tensor("out", [ROWS, F], dt, kind="ExternalOutput")

    x_t = x.rearrange("(n p) f -> n p f", p=P)
    o_t = out.rearrange("(n p) f -> n p f", p=P)

    n_iters = reps * N_TILES

    with contextlib.ExitStack() as ctx:
        m_b = ctx.enter_context(nc.sbuf_tensor("m_b", [P, F], dt))
        mprime = ctx.enter_context(nc.sbuf_tensor("mprime", [P, F], dt))
        xt = ctx.enter_context(nc.sbuf_tensor("xt", [P, NBUF, F], dt))
        s_med = ctx.enter_context(nc.semaphore("s_med"))
        s_bc = ctx.enter_context(nc.semaphore("s_bc"))
        s_fan = ctx.enter_context(nc.semaphore("s_fan"))
        s_ldh = [ctx.enter_context(nc.semaphore(f"s_ldh{h}")) for h in range(2)]
        s_sth = [ctx.enter_context(nc.semaphore(f"s_sth{h}")) for h in range(2)]
        s_ld = [ctx.enter_context(nc.semaphore(f"s_ld{s}")) for s in range(NBUF)]
        s_st = [ctx.enter_context(nc.semaphore(f"s_st{s}")) for s in range(NBUF)]
        s_dve = ctx.enter_context(nc.semaphore("s_dve"))
        block = ctx.enter_context(nc.Block())

        # s_dve counts: +1 per mprime prep op (2), then +1 per TT_i,
        # so after TT_i the value is i + 3.

        F2 = F // 2

        @block.sync
        def _(sync):
            for i in range(n_iters):
                s = i % NBUF
                if i >= NBUF:
                    # overwriting xt[:, s]: store_{i-NBUF} done implies
                    # TT_{i-NBUF} done as well
                    sync.wait_ge(s_st[s], 16 * (i // NBUF))
                if i < n_iters - 1:
                    sync.dma_start(out=xt[:, s], in_=x_t[i % N_TILES]).then_inc(
                        s_ld[s], 16
                    )
                else:
                    # globally-last tile: two independent column halves so
                    # compute/store overlap the tail of the final load
                    sync.dma_start(
                        out=xt[:, s][:, :F2], in_=x_t[i % N_TILES][:, :F2]
                    ).then_inc(s_ldh[0], 16)
                    sync.dma_start(
                        out=xt[:, s][:, F2:], in_=x_t[i % N_TILES][:, F2:]
                    ).then_inc(s_ldh[1], 16)

        @block.scalar
        def _(scalar):
            # 16 KB medians row -> partition 0; prep runs on that row, then
            # log2 doubling copies spread mprime row 0 across all 128
            # partitions SBUF->SBUF (only 16 KB of HBM read instead of the
            # 2 MiB a DRAM-side broadcast would re-read)
            scalar.dma_start(out=m_b[:1, :], in_=med[None, :]).then_inc(s_med, 16)
            scalar.wait_ge(s_dve, 2)  # mprime[0:1, :] final
            # double serially up to 16 partitions...
            k, chain = 1, 0
            while k < 16:
                scalar.dma_start(
                    out=mprime[k : 2 * k, :], in_=mprime[:k, :]
                ).then_inc(s_bc, 16)
                chain += 1
                scalar.wait_ge(s_bc, 16 * chain)
                k *= 2
            # ...then fan out the remaining 7 copies concurrently (same
            # source, disjoint dests); s_fan is only ever waited at the sum
            for j in range(1, 8):
                scalar.dma_start(
                    out=mprime[16 * j : 16 * (j + 1), :], in_=mprime[:16, :]
                ).then_inc(s_fan, 16)
            for i in range(n_iters):
                s = i % NBUF
                if i < n_iters - 1:
                    scalar.wait_ge(s_dve, i + 3)  # TT_i rewrote xt[:, s]
                    scalar.dma_start(out=o_t[i % N_TILES], in_=xt[:, s]).then_inc(
                        s_st[s], 16
                    )
                else:
                    scalar.wait_ge(s_dve, i + 3)  # TT on first half done
                    scalar.dma_start(
                        out=o_t[i % N_TILES][:, :F2], in_=xt[:, s][:, :F2]
                    ).then_inc(s_sth[0], 16)
                    scalar.wait_ge(s_dve, i + 4)  # TT on second half done
                    scalar.dma_start(
                        out=o_t[i % N_TILES][:, F2:], in_=xt[:, s][:, F2:]
                    ).then_inc(s_sth[1], 16)
            # all stores landed before the NEFF retires
            if n_iters:
                for s in range(NBUF):
                    n_full = sum(
                        1 for t in range(n_iters - 1) if t % NBUF == s
                    )
                    if n_full:
                        scalar.wait_ge(s_st[s], 16 * n_full)
                scalar.wait_ge(s_sth[0], 16)
                scalar.wait_ge(s_sth[1], 16)

        @block.vector
        def _(vector):
            vector.wait_ge(s_med, 16)  # medians row present
            # mprime = (med <= 0) * BIG + med, on partition 0 only; sem
            # handshakes order the back-to-back DVE ops (same-engine RAW is
            # not implicit)
            nc.vector.tensor_scalar(
                out=mprime[:1, :],
                in0=m_b[:1, :],
                scalar1=0.0,
                scalar2=_BIG,
                op0=mybir.AluOpType.is_le,
                op1=mybir.AluOpType.mult,
            ).then_inc(s_dve, 1)
            vector.wait_ge(s_dve, 1)
            nc.vector.tensor_add(
                out=mprime[:1, :], in0=mprime[:1, :], in1=m_b[:1, :]
            ).then_inc(s_dve, 1)
            vector.wait_ge(s_fan, 16 * 7)  # all 7 fan-out copies landed
            for i in range(n_iters):
                s = i % NBUF
                if i >= NBUF:
                    # in-place overwrite of xt[:, s] must wait until
                    # store_{i-NBUF} has read it
                    vector.wait_ge(s_st[s], 16 * (i // NBUF))
                if i < n_iters - 1:
                    vector.wait_ge(s_ld[s], 16 * (i // NBUF + 1))  # loaded
                    nc.vector.tensor_tensor(
                        out=xt[:, s], in0=xt[:, s], in1=mprime[:],
                        op=mybir.AluOpType.is_ge,
                    ).then_inc(s_dve, 1)
                else:
                    vector.wait_ge(s_ldh[0], 16)
                    nc.vector.tensor_tensor(
                        out=xt[:, s][:, :F2], in0=xt[:, s][:, :F2],
                        in1=mprime[:, :F2], op=mybir.AluOpType.is_ge,
                    ).then_inc(s_dve, 1)
                    vector.wait_ge(s_ldh[1], 16)
                    nc.vector.tensor_tensor(
                        out=xt[:, s][:, F2:], in0=xt[:, s][:, F2:],
                        in1=mprime[:, F2:], op=mybir.AluOpType.is_ge,
                    ).then_inc(s_dve, 1)

    return nc


_NC_CACHE: list[bass.Bass] = []


def _get_nc() -> bass.Bass:
    if not _NC_CACHE:
        _NC_CACHE.append(_build_nc())
    return _NC_CACHE[0]


def kernel(x: np.ndarray, medians: np.ndarray) -> np.ndarray:
    x = np.ascontiguousarray(x, dtype=np.float32)
    medians = np.ascontiguousarray(medians, dtype=np.float32)
    assert x.shape == (B_FULL, F), x.shape
    assert medians.shape == (F,), medians.shape

    nc = _get_nc()
    in_maps = [
        {"x": x[c * ROWS : (c + 1) * ROWS], "med": medians} for c in range(N_CORES)
    ]
    res = run_bass_kernel_spmd(nc, in_maps, core_ids=list(range(N_CORES)))
    return np.concatenate([res.results[c]["out"] for c in range(N_CORES)], axis=0)


# revision 28
# speedup vs baseline: 1.0437x; 1.0066x over previous
"""Binarize kernel for Trainium2, 8-core data-parallel.

out[b, f] = 1.0 if (medians[f] > 0) and (x[b, f] >= medians[f]) else 0.0

Sharding: pure data parallel - x is split row-wise across the 8 NeuronCores
(2048 rows each); the 4096-entry medians vector is replicated.

Per-core device kernel (raw bass, three engine streams):
  * ACT ring: broadcast-DMA medians across the 128 partitions once, then
    stream the 16 output tiles back to HBM.
  * SP ring: stream the 16 [128, 4096] x tiles HBM->SBUF (starts at t=0).
  * DVE: mprime[f] = medians[f] if medians[f] > 0 else 3e38 (two prep ops),
    then one in-place is_ge compare per tile: xt = (xt >= mprime) -> 1.0/0.0.
    A single compare is exact - no arithmetic rounding anywhere.

Raw bass instead of the Tile framework because walrus codegen allows only a
single sync-wait command on a compute instruction; all waits here are
standalone queue commands. Each of the NBUF=8 buffer slots has its own
load/store semaphore pair: increments on one semaphore are serialized by the
slot's dependency chain, so count thresholds are race-free even though DMA
completions across slots may reorder. The kernel is HBM-bound: ~64 MiB of
HBM traffic per core at ~330 GB/s measured => ~205 us steady-state.

reps > 1 re-runs the identical pipeline inside one NEFF (slope-based HW
timing); the output is unchanged.
"""

import contextlib

import numpy as np

import concourse.bass as bass
import concourse.mybir as mybir
from concourse.bass_utils import run_bass_kernel_spmd

N_CORES = 8
B_FULL = 16384
F = 4096
ROWS = B_FULL // N_CORES  # 2048 rows per core
P = 128
N_TILES = ROWS // P  # 16
NBUF = 8

_BIG = 3.0e38  # pushes the compare threshold above any finite fp32 input


def _build_nc(reps: int = 1) -> bass.Bass:
    nc = bass.Bass()
    dt = mybir.dt.float32
    x = nc.dram_tensor("x", [ROWS, F], dt, kind="ExternalInput")
    med = nc.dram_tensor("med", [F], dt, kind="ExternalInput")
    out = nc.dram_tensor("out", [ROWS, F], dt, kind="ExternalOutput")

    x_t = x.rearrange("(n p) f -> n p f", p=P)
    o_t = out.rearrange("(n p) f -> n p f", p=P)

    n_iters = reps * N_TILES

    with contextlib.ExitStack() as ctx:
        m_b = ctx.enter_context(nc.sbuf_tensor("m_b", [P, F], dt))
        mprime = ctx.enter_context(nc.sbuf_tensor("mprime", [P, F], dt))
        xt = ctx.enter_context(nc.sbuf_tensor("xt", [P, NBUF, F], dt))
        s_med = ctx.enter_context(nc.semaphore("s_med"))
        s_bc = ctx.enter_context(nc.semaphore("s_bc"))
        s_fan = ctx.enter_context(nc.semaphore("s_fan"))
        NSPLIT = 4  # the globally-last tile is processed in NSPLIT col-pieces
        s_ldh = [ctx.enter_context(nc.semaphore(f"s_ldh{h}")) for h in range(NSPLIT)]
        s_sth = [ctx.enter_context(nc.semaphore(f"s_sth{h}")) for h in range(NSPLIT)]
        s_ld = [ctx.enter_context(nc.semaphore(f"s_ld{s}")) for s in range(NBUF)]
        s_st = [ctx.enter_context(nc.semaphore(f"s_st{s}")) for s in range(NBUF)]
        s_dve = ctx.enter_context(nc.semaphore("s_dve"))
        block = ctx.enter_context(nc.Block())

        # s_dve counts: +1 per mprime prep op (2), then +1 per TT_i,
        # so after TT_i the value is i + 3.

        F2 = F // 2

        @block.sync
        def _(sync):
            for i in range(n_iters):
                s = i % NBUF
                if i >= NBUF:
                    # overwriting xt[:, s]: store_{i-NBUF} done implies
                    # TT_{i-NBUF} done as well
                    sync.wait_ge(s_st[s], 16 * (i // NBUF))
                if i < n_iters - 1:
                    sync.dma_start(out=xt[:, s], in_=x_t[i % N_TILES]).then_inc(
                        s_ld[s], 16
                    )
                else:
                    # globally-last tile: NSPLIT independent column pieces so
                    # compute/store overlap the tail of the final load
                    for h in range(NSPLIT):
                        c0, c1 = h * (F // NSPLIT), (h + 1) * (F // NSPLIT)
                        sync.dma_start(
                            out=xt[:, s][:, c0:c1],
                            in_=x_t[i % N_TILES][:, c0:c1],
                        ).then_inc(s_ldh[h], 16)

        @block.scalar
        def _(scalar):
            # 16 KB medians row -> partition 0; prep runs on that row, then
            # log2 doubling copies spread mprime row 0 across all 128
            # partitions SBUF->SBUF (only 16 KB of HBM read instead of the
            # 2 MiB a DRAM-side broadcast would re-read)
            scalar.dma_start(out=m_b[:1, :], in_=med[None, :]).then_inc(s_med, 16)
            scalar.wait_ge(s_dve, 2)  # mprime[0:1, :] final
            # double serially up to 16 partitions...
            k, chain = 1, 0
            while k < 16:
                scalar.dma_start(
                    out=mprime[k : 2 * k, :], in_=mprime[:k, :]
                ).then_inc(s_bc, 16)
                chain += 1
                scalar.wait_ge(s_bc, 16 * chain)
                k *= 2
            # ...then fan out the remaining 7 copies concurrently (same
            # source, disjoint dests); s_fan is only ever waited at the sum
            for j in range(1, 8):
                scalar.dma_start(
                    out=mprime[16 * j : 16 * (j + 1), :], in_=mprime[:16, :]
                ).then_inc(s_fan, 16)
            for i in range(n_iters):
                s = i % NBUF
                if i < n_iters - 1:
                    scalar.wait_ge(s_dve, i + 3)  # TT_i rewrote xt[:, s]
                    scalar.dma_start(out=o_t[i % N_TILES], in_=xt[:, s]).then_inc(
                        s_st[s], 16
                    )
                else:
                    for h in range(NSPLIT):
                        c0, c1 = h * (F // NSPLIT), (h + 1) * (F // NSPLIT)
                        scalar.wait_ge(s_dve, i + 3 + h)  # TT on piece h done
                        scalar.dma_start(
                            out=o_t[i % N_TILES][:, c0:c1],
                            in_=xt[:, s][:, c0:c1],
                        ).then_inc(s_sth[h], 16)
            # all stores landed before the NEFF retires
            if n_iters:
                for s in range(NBUF):
                    n_full = sum(
                        1 for t in range(n_iters - 1) if t % NBUF == s
                    )
                    if n_full:
                        scalar.wait_ge(s_st[s], 16 * n_full)
                for h in range(NSPLIT):
                    scalar.wait_ge(s_sth[h], 16)

        @block.vector
        def _(vector):
            vector.wait_ge(s_med, 16)  # medians row present
            # mprime = (med <= 0) * BIG + med, on partition 0 only; sem
            # handshakes order the back-to-back DVE ops (same-engine RAW is
            # not implicit)
            nc.vector.tensor_scalar(
                out=mprime[:1, :],
                in0=m_b[:1, :],
                scalar1=0.0,
                scalar2=_BIG,
                op0=mybir.AluOpType.is_le,
                op1=mybir.AluOpType.mult,
            ).then_inc(s_dve, 1)
            vector.wait_ge(s_dve, 1)
            nc.vector.tensor_add(
                out=mprime[:1, :], in0=mprime[:1, :], in1=m_b[:1, :]
            ).then_inc(s_dve, 1)
            vector.wait_ge(s_fan, 16 * 7)  # all 7 fan-out copies landed
            for i in range(n_iters):
                s = i % NBUF
                if i >= NBUF:
                    # in-place overwrite of xt[:, s] must wait until
                    # store_{i-NBUF} has read it
                    vector.wait_ge(s_st[s], 16 * (i // NBUF))
                if i < n_iters - 1:
                    vector.wait_ge(s_ld[s], 16 * (i // NBUF + 1))  # loaded
                    nc.vector.tensor_tensor(
                        out=xt[:, s], in0=xt[:, s], in1=mprime[:],
                        op=mybir.AluOpType.is_ge,
                    ).then_inc(s_dve, 1)
                else:
                    for h in range(NSPLIT):
                        c0, c1 = h * (F // NSPLIT), (h + 1) * (F // NSPLIT)
                        vector.wait_ge(s_ldh[h], 16)
                        nc.vector.tensor_tensor(
                            out=xt[:, s][:, c0:c1], in0=xt[:, s][:, c0:c1],
                            in1=mprime[:, c0:c1], op=mybir.AluOpType.is_ge,
                        ).then_inc(s_dve, 1)

    return nc


_NC_CACHE: list[bass.Bass] = []


def _get_nc() -> bass.Bass:
    if not _NC_CACHE:
        _NC_CACHE.append(_build_nc())
    return _NC_CACHE[0]


def kernel(x: np.ndarray, medians: np.ndarray) -> np.ndarray:
    x = np.ascontiguousarray(x, dtype=np.float32)
    medians = np.ascontiguousarray(medians, dtype=np.float32)
    assert x.shape == (B_FULL, F), x.shape
    assert medians.shape == (F,), medians.shape

    nc = _get_nc()
    in_maps = [
        {"x": x[c * ROWS : (c + 1) * ROWS], "med": medians} for c in range(N_CORES)
    ]
    res = run_bass_kernel_spmd(nc, in_maps, core_ids=list(range(N_CORES)))
    return np.concatenate([res.results[c]["out"] for c in range(N_CORES)], axis=0)


# revision 29
# speedup vs baseline: 1.0727x; 1.0278x over previous
"""Binarize kernel for Trainium2, 8-core data-parallel.

out[b, f] = 1.0 if (medians[f] > 0) and (x[b, f] >= medians[f]) else 0.0

Sharding: pure data parallel - x is split row-wise across the 8 NeuronCores
(2048 rows each); the 4096-entry medians vector is replicated.

Per-core device kernel (raw bass, three engine streams):
  * ACT ring: broadcast-DMA medians across the 128 partitions once, then
    stream the 16 output tiles back to HBM.
  * SP ring: stream the 16 [128, 4096] x tiles HBM->SBUF (starts at t=0).
  * DVE: mprime[f] = medians[f] if medians[f] > 0 else 3e38 (two prep ops),
    then one in-place is_ge compare per tile: xt = (xt >= mprime) -> 1.0/0.0.
    A single compare is exact - no arithmetic rounding anywhere.

Raw bass instead of the Tile framework because walrus codegen allows only a
single sync-wait command on a compute instruction; all waits here are
standalone queue commands. Each of the NBUF=8 buffer slots has its own
load/store semaphore pair: increments on one semaphore are serialized by the
slot's dependency chain, so count thresholds are race-free even though DMA
completions across slots may reorder. The kernel is HBM-bound: ~64 MiB of
HBM traffic per core at ~330 GB/s measured => ~205 us steady-state.

reps > 1 re-runs the identical pipeline inside one NEFF (slope-based HW
timing); the output is unchanged.
"""

import contextlib

import numpy as np

import concourse.bass as bass
import concourse.mybir as mybir
from concourse.bass_utils import run_bass_kernel_spmd

N_CORES = 8
B_FULL = 16384
F = 4096
ROWS = B_FULL // N_CORES  # 2048 rows per core
P = 128
N_TILES = ROWS // P  # 16
NBUF = 8

_BIG = 3.0e38  # pushes the compare threshold above any finite fp32 input


def _build_nc(reps: int = 1) -> bass.Bass:
    nc = bass.Bass()
    dt = mybir.dt.float32
    x = nc.dram_tensor("x", [ROWS, F], dt, kind="ExternalInput")
    med = nc.dram_tensor("med", [F], dt, kind="ExternalInput")
    out = nc.dram_tensor("out", [ROWS, F], dt, kind="ExternalOutput")

    x_t = x.rearrange("(n p) f -> n p f", p=P)
    o_t = out.rearrange("(n p) f -> n p f", p=P)

    n_iters = reps * N_TILES

    with contextlib.ExitStack() as ctx:
        m_b = ctx.enter_context(nc.sbuf_tensor("m_b", [P, F], dt))
        mprime = ctx.enter_context(nc.sbuf_tensor("mprime", [P, F], dt))
        xt = ctx.enter_context(nc.sbuf_tensor("xt", [P, NBUF, F], dt))
        s_med = ctx.enter_context(nc.semaphore("s_med"))
        s_bc = ctx.enter_context(nc.semaphore("s_bc"))
        s_fan = ctx.enter_context(nc.semaphore("s_fan"))
        NSPLIT = 8  # the globally-last tile is processed in NSPLIT col-pieces
        s_ldh = [ctx.enter_context(nc.semaphore(f"s_ldh{h}")) for h in range(NSPLIT)]
        s_sth = [ctx.enter_context(nc.semaphore(f"s_sth{h}")) for h in range(NSPLIT)]
        s_ld = [ctx.enter_context(nc.semaphore(f"s_ld{s}")) for s in range(NBUF)]
        s_st = [ctx.enter_context(nc.semaphore(f"s_st{s}")) for s in range(NBUF)]
        s_dve = ctx.enter_context(nc.semaphore("s_dve"))
        block = ctx.enter_context(nc.Block())

        # s_dve counts: +1 per mprime prep op (2), then +1 per TT_i,
        # so after TT_i the value is i + 3.

        F2 = F // 2

        @block.sync
        def _(sync):
            for i in range(n_iters):
                s = i % NBUF
                if i >= NBUF:
                    # overwriting xt[:, s]: store_{i-NBUF} done implies
                    # TT_{i-NBUF} done as well
                    sync.wait_ge(s_st[s], 16 * (i // NBUF))
                if i < n_iters - 1:
                    sync.dma_start(out=xt[:, s], in_=x_t[i % N_TILES]).then_inc(
                        s_ld[s], 16
                    )
                else:
                    # globally-last tile: NSPLIT independent column pieces so
                    # compute/store overlap the tail of the final load
                    for h in range(NSPLIT):
                        c0, c1 = h * (F // NSPLIT), (h + 1) * (F // NSPLIT)
                        sync.dma_start(
                            out=xt[:, s][:, c0:c1],
                            in_=x_t[i % N_TILES][:, c0:c1],
                        ).then_inc(s_ldh[h], 16)

        @block.scalar
        def _(scalar):
            # 16 KB medians row -> partition 0; prep runs on that row, then
            # log2 doubling copies spread mprime row 0 across all 128
            # partitions SBUF->SBUF (only 16 KB of HBM read instead of the
            # 2 MiB a DRAM-side broadcast would re-read)
            scalar.dma_start(out=m_b[:1, :], in_=med[None, :]).then_inc(s_med, 16)
            scalar.wait_ge(s_dve, 2)  # mprime[0:1, :] final
            # double serially up to 16 partitions...
            k, chain = 1, 0
            while k < 16:
                scalar.dma_start(
                    out=mprime[k : 2 * k, :], in_=mprime[:k, :]
                ).then_inc(s_bc, 16)
                chain += 1
                scalar.wait_ge(s_bc, 16 * chain)
                k *= 2
            # ...then fan out the remaining 7 copies concurrently (same
            # source, disjoint dests); s_fan is only ever waited at the sum
            for j in range(1, 8):
                scalar.dma_start(
                    out=mprime[16 * j : 16 * (j + 1), :], in_=mprime[:16, :]
                ).then_inc(s_fan, 16)
            for i in range(n_iters):
                s = i % NBUF
                if i < n_iters - 1:
                    scalar.wait_ge(s_dve, i + 3)  # TT_i rewrote xt[:, s]
                    scalar.dma_start(out=o_t[i % N_TILES], in_=xt[:, s]).then_inc(
                        s_st[s], 16
                    )
                else:
                    for h in range(NSPLIT):
                        c0, c1 = h * (F // NSPLIT), (h + 1) * (F // NSPLIT)
                        scalar.wait_ge(s_dve, i + 3 + h)  # TT on piece h done
                        scalar.dma_start(
                            out=o_t[i % N_TILES][:, c0:c1],
                            in_=xt[:, s][:, c0:c1],
                        ).then_inc(s_sth[h], 16)
            # all stores landed before the NEFF retires
            if n_iters:
                for s in range(NBUF):
                    n_full = sum(
                        1 for t in range(n_iters - 1) if t % NBUF == s
                    )
                    if n_full:
                        scalar.wait_ge(s_st[s], 16 * n_full)
                for h in range(NSPLIT):
                    scalar.wait_ge(s_sth[h], 16)

        @block.vector
        def _(vector):
            vector.wait_ge(s_med, 16)  # medians row present
            # mprime = (med <= 0) * BIG + med, on partition 0 only; sem
            # handshakes order the back-to-back DVE ops (same-engine RAW is
            # not implicit)
            nc.vector.tensor_scalar(
                out=mprime[:1, :],
                in0=m_b[:1, :],
                scalar1=0.0,
                scalar2=_BIG,
                op0=mybir.AluOpType.is_le,
                op1=mybir.AluOpType.mult,
            ).then_inc(s_dve, 1)
            vector.wait_ge(s_dve, 1)
            nc.vector.tensor_add(
                out=mprime[:1, :], in0=mprime[:1, :], in1=m_b[:1, :]
            ).then_inc(s_dve, 1)
            vector.wait_ge(s_fan, 16 * 7)  # all 7 fan-out copies landed
            for i in range(n_iters):
                s = i % NBUF
                if i >= NBUF:
                    # in-place overwrite of xt[:, s] must wait until
                    # store_{i-NBUF} has read it
                    vector.wait_ge(s_st[s], 16 * (i // NBUF))
                if i < n_iters - 1:
                    vector.wait_ge(s_ld[s], 16 * (i // NBUF + 1))  # loaded
                    nc.vector.tensor_tensor(
                        out=xt[:, s], in0=xt[:, s], in1=mprime[:],
                        op=mybir.AluOpType.is_ge,
                    ).then_inc(s_dve, 1)
                else:
                    for h in range(NSPLIT):
                        c0, c1 = h * (F // NSPLIT), (h + 1) * (F // NSPLIT)
                        vector.wait_ge(s_ldh[h], 16)
                        nc.vector.tensor_tensor(
                            out=xt[:, s][:, c0:c1], in0=xt[:, s][:, c0:c1],
                            in1=mprime[:, c0:c1], op=mybir.AluOpType.is_ge,
                        ).then_inc(s_dve, 1)

    return nc


_NC_CACHE: list[bass.Bass] = []


def _get_nc() -> bass.Bass:
    if not _NC_CACHE:
        _NC_CACHE.append(_build_nc())
    return _NC_CACHE[0]


def kernel(x: np.ndarray, medians: np.ndarray) -> np.ndarray:
    x = np.ascontiguousarray(x, dtype=np.float32)
    medians = np.ascontiguousarray(medians, dtype=np.float32)
    assert x.shape == (B_FULL, F), x.shape
    assert medians.shape == (F,), medians.shape

    nc = _get_nc()
    in_maps = [
        {"x": x[c * ROWS : (c + 1) * ROWS], "med": medians} for c in range(N_CORES)
    ]
    res = run_bass_kernel_spmd(nc, in_maps, core_ids=list(range(N_CORES)))
    return np.concatenate([res.results[c]["out"] for c in range(N_CORES)], axis=0)
